# revision 1
# baseline (speedup 1.0000x reference)
"""Adaptive Spectral Block on 8 Trainium2 NeuronCores.

Strategy: data-parallel over batch (1 sample/core). Two device launches:
  L1: four-step radix-64 forward rfft (fp32 matmuls) + per-frequency energy
  host: quantile thresholds (tiny: 8x2049 values) -> mask scale vectors
  L2: block-diag complex MLP + softshrink + spectral combine + four-step irfft
The mid-FFT transpose is routed through DRAM scratch with large affine DMAs.
"""

import math
import time as _time
import numpy as np

import concourse.bass as bass
import concourse.tile as tile
from concourse import bacc, mybir
from concourse.bass_utils import run_bass_kernel_spmd

F32 = mybir.dt.float32
BF16 = mybir.dt.bfloat16
FR = mybir.dt.float32r
AX = mybir.AxisListType
ALU = mybir.AluOpType
ACTF = mybir.ActivationFunctionType

B, N, C = 8, 4096, 768
R = 64            # radix
F = N // 2 + 1    # 2049
K1Q = 33          # inverse stage-1 contraction length (2112 = 33*64 padded spectrum)
FP = 2112         # padded spectrum length
NBLK, BS = 8, 96  # MLP blocks
LAMBD = 0.01
LOW_Q = 0.5

_CACHE = {}
TRACE = False
LAST_NS = []


# ------------------------------------------------------------------ matrices
def _fwd_mats():
    n1 = np.arange(R)
    DC = np.cos(2 * np.pi * np.outer(n1, n1) / R)
    DS = -np.sin(2 * np.pi * np.outer(n1, n1) / R)
    dstack = (np.concatenate([DC, DS], axis=1) / 64.0).astype(np.float32)  # (64,128)

    k2 = np.arange(R)
    n2 = np.arange(R)
    tm = np.zeros((R, 128, 128), np.float32)
    for q in range(R):
        ang = 2 * np.pi * (np.outer(k2, n2 * 64) + n2[None, :] * q) / N
        TR, TI = np.cos(ang), -np.sin(ang)
        tm[q, :64, :64] = TR.T
        tm[q, 64:, :64] = -TI.T
        tm[q, :64, 64:] = TI.T
        tm[q, 64:, 64:] = TR.T
    # pre-arranged for SBUF (p, (q, m)) layout
    tmats = np.ascontiguousarray(tm.transpose(1, 0, 2)).reshape(128, R * 128)
    return dstack, tmats


def _inv_mats():
    b0 = np.arange(R)
    k1q = np.arange(K1Q)
    VC = np.cos(2 * np.pi * np.outer(k1q, b0) / R)
    VS = np.sin(2 * np.pi * np.outer(k1q, b0) / R)
    vstack = np.zeros((66, 128), np.float32)
    vstack[:33, :64] = VC
    vstack[33:, :64] = -VS
    vstack[:33, 64:] = VS
    vstack[33:, 64:] = VC
    vstack *= 2.0 / 64.0

    b1 = np.arange(R)
    k0 = np.arange(R)
    mm = np.zeros((R, 128, 64), np.float32)
    for q in range(R):
        ang = 2 * np.pi * (np.outer(b1, k0 * 64) + k0[None, :] * q) / N
        mm[q, :64] = np.cos(ang).T
        mm[q, 64:] = -np.sin(ang).T
    mmats = np.ascontiguousarray(mm.transpose(1, 0, 2)).reshape(128, R * 64)
    return vstack, mmats


# ------------------------------------------------------------------ launch 1
def _build_l1():
    dstack_np, tmats_np = _fwd_mats()
    nc = bacc.Bacc(None, target_bir_lowering=False)
    x = nc.dram_tensor("x", [N, C], F32, kind="ExternalInput")
    xf_re = nc.dram_tensor("xf_re", [33, 64, C], F32, kind="ExternalOutput")
    xf_im = nc.dram_tensor("xf_im", [33, 64, C], F32, kind="ExternalOutput")
    energy = nc.dram_tensor("energy", [128, 64], F32, kind="ExternalOutput")
    dstack_h = nc.inline_tensor(dstack_np, name="dstack")
    tmats_h = nc.inline_tensor(tmats_np, name="tmats")

    with tile.TileContext(nc) as tc:
        with (
            tc.tile_pool(name="const", bufs=1) as constp,
            tc.tile_pool(name="en", bufs=1) as enp,
            tc.tile_pool(name="dram", bufs=1, space="DRAM") as dramp,
        ):
            dsb = constp.tile([64, 128], F32)
            nc.sync.dma_start(dsb[:], dstack_h[:])
            tsb = constp.tile([128, R * 128], F32)
            nc.sync.dma_start(tsb[:], tmats_h[:])
            en_acc = enp.tile([128, 64], F32)

            ya = dramp.tile([128, 64, C], F32)  # [k1stack, n2, c] scratch

            x3 = x[:].rearrange("(a b) c -> a b c", b=R)  # (n1, n2, c)
            with (
                tc.tile_pool(name="xin", bufs=3) as xp,
                tc.tile_pool(name="psA", bufs=3, space="PSUM") as psA,
            ):
                for n2 in range(R):
                    xt = xp.tile([64, C], F32, name="xt")
                    nc.sync.dma_start(xt[:], x3[:, n2, :])
                    ps = psA.tile([128, C], F32, name="psA")
                    nc.tensor.matmul(ps[:, 0:512], dsb[:], xt[:, 0:512],
                                     start=True, stop=True)
                    nc.tensor.matmul(ps[:, 512:768], dsb[:], xt[:, 512:768],
                                     start=True, stop=True)
                    ev = xp.tile([128, C], F32, name="ev")
                    if n2 % 2 == 0:
                        nc.scalar.copy(ev[:], ps[:])
                    else:
                        nc.vector.tensor_copy(ev[:], ps[:])
                    nc.sync.dma_start(ya[:, n2, :], ev[:])

            tc.strict_bb_all_engine_barrier()
            with (
                tc.tile_pool(name="zt", bufs=3) as ztp,
                tc.tile_pool(name="psB", bufs=3, space="PSUM") as psB,
                tc.tile_pool(name="sq", bufs=3) as sqp,
            ):
                for q in range(R):
                    zt = ztp.tile([128, C], F32, name="zt")
                    nc.sync.dma_start(zt[0:64, :], ya[q, :, :])
                    nc.sync.dma_start(zt[64:128, :], ya[64 + q, :, :])
                    ps = psB.tile([128, C], F32, name="psB")
                    lhs = tsb[:, q * 128:(q + 1) * 128]
                    nc.tensor.matmul(ps[:, 0:512], lhs, zt[:, 0:512],
                                     start=True, stop=True)
                    nc.tensor.matmul(ps[:, 512:768], lhs, zt[:, 512:768],
                                     start=True, stop=True)
                    sq = sqp.tile([128, C], F32, name="sq")
                    nc.scalar.activation(sq[:], ps[:], ACTF.Square,
                                         accum_out=en_acc[:, q:q + 1])
                    kmax = 33 if q == 0 else 32
                    ev = sqp.tile([128, C], F32, name="evB")
                    nc.vector.tensor_copy(ev[:], ps[:])
                    nc.sync.dma_start(xf_re[0:kmax, q, :], ev[0:kmax, :])
                    nc.sync.dma_start(xf_im[0:kmax, q, :], ev[64:64 + kmax, :])

            nc.sync.dma_start(energy[:], en_acc[:])
    nc.compile()
    return nc


# ------------------------------------------------------------------ launch 2
def _build_l2():
    import ml_dtypes
    vstack_np, mmats_np = _inv_mats()
    vstack_np_bf16 = vstack_np.astype(ml_dtypes.bfloat16)
    mmats_np_bf16 = mmats_np.astype(ml_dtypes.bfloat16)
    nc = bacc.Bacc(None, target_bir_lowering=False)
    xfreT = nc.dram_tensor("xfreT", [C, F], F32, kind="ExternalInput")
    xfimT = nc.dram_tensor("xfimT", [C, F], F32, kind="ExternalInput")
    # m-scale vectors (per-frequency): re: u_r = P*mv1 + Q*mv2 ; im: u_i = Pi*mv3 + Qi*mv4
    mvs = nc.dram_tensor("mvs", [1, F], F32, kind="ExternalInput")
    # weights pre-arranged: (96, 8*96) blocks of lhsT
    w1r = nc.dram_tensor("w1r", [BS, NBLK * BS], BF16, kind="ExternalInput")
    w1i = nc.dram_tensor("w1i", [BS, NBLK * BS], BF16, kind="ExternalInput")
    w1in = nc.dram_tensor("w1in", [BS, NBLK * BS], BF16, kind="ExternalInput")
    w2r = nc.dram_tensor("w2r", [BS, NBLK * BS], BF16, kind="ExternalInput")
    w2i = nc.dram_tensor("w2i", [BS, NBLK * BS], BF16, kind="ExternalInput")
    w2in = nc.dram_tensor("w2in", [BS, NBLK * BS], BF16, kind="ExternalInput")
    # biases / vectors packed (96, 8): col k
    bvec = nc.dram_tensor("bvec", [BS, 8 * NBLK], F32, kind="ExternalInput")
    # bvec cols: [b1r, b1i, b2rm, b2rn, b2im, b2in, _, _] interleaved per k? ->
    # layout: bvec[:, t*NBLK + k] for t in 0..7
    wvec = nc.dram_tensor("wvec", [BS, 4 * NBLK], F32, kind="ExternalInput")
    # wvec cols: [wr, wi, whr, whi] x k
    out = nc.dram_tensor("out", [64, 64, C], F32, kind="ExternalOutput")
    vstack_h = nc.inline_tensor(vstack_np.astype(np.dtype("bfloat16") if False else None) if False else vstack_np_bf16, name="vstack")
    mmats_h = nc.inline_tensor(mmats_np_bf16, name="mmats")

    MMCH = [(0, 512), (512, 512), (1024, 512), (1536, 512), (2048, 1)]

    with tile.TileContext(nc) as tc:
        with (
            tc.tile_pool(name="const", bufs=1) as constp,
            tc.tile_pool(name="dram", bufs=1, space="DRAM") as dramp,
        ):
            wsb = {}
            for nm, h in [("w1r", w1r), ("w1i", w1i), ("w1in", w1in),
                          ("w2r", w2r), ("w2i", w2i), ("w2in", w2in)]:
                t = constp.tile([BS, NBLK * BS], BF16, name=nm)
                nc.sync.dma_start(t[:], h[:])
                wsb[nm] = t
            bsb = constp.tile([BS, 8 * NBLK], F32)
            nc.sync.dma_start(bsb[:], bvec[:])
            wvb = constp.tile([BS, 4 * NBLK], F32)
            nc.sync.dma_start(wvb[:], wvec[:])
            vsb = constp.tile([66, 128], BF16)
            nc.sync.dma_start(vsb[:], vstack_h[:])
            msb = constp.tile([128, R * 64], BF16)
            nc.sync.dma_start(msb[:], mmats_h[:])

            ubuf = dramp.tile([2, C, FP], BF16)   # [re/im, c, f]
            u1buf = dramp.tile([128, R, C], BF16)  # [b0stack, k0, c]

            # ---- m-vector broadcast to (96, F) via K=1 matmul ----
            onesb = constp.tile([1, BS], F32)
            nc.vector.memset(onesb[:], 1.0)
            mbc = constp.tile([BS, F], BF16, name="mbc")
            with (
                tc.tile_pool(name="mvstage", bufs=1) as mvp,
                tc.tile_pool(name="psm", bufs=4, space="PSUM") as psm,
            ):
                mvsb = mvp.tile([1, F], F32, name="mvs")
                nc.sync.dma_start(mvsb[:], mvs[0, :])
                for o, w in [(0, 512), (512, 512), (1024, 512),
                             (1536, 512), (2048, 1)]:
                    pst = psm.tile([BS, 512], F32, name="psb")
                    nc.tensor.matmul(pst[:, 0:w], onesb[:],
                                     mvsb[:, o:o + w],
                                     start=True, stop=True)
                    nc.scalar.copy(mbc[:, o:o + w], pst[:, 0:w])

            # ---- zero the spectrum pad f in [2049, 2112) ----
            zpad = constp.tile([128, FP - F], BF16)
            nc.vector.memset(zpad[:], 0.0)
            ub2 = ubuf[:].rearrange("h c f -> (h c) f")
            zpad_dmas = []
            for j in range(2 * C // 128):
                zpad_dmas.append(
                    nc.sync.dma_start(ub2[j * 128:(j + 1) * 128, F:FP], zpad[:]))
            from concourse.tile import add_dep_helper
            ub4 = ubuf[:].rearrange("h c (k1 k0) -> h k1 c k0", k0=R)
            funnels = {}

            # ---- per-block MLP + combine ----
            with (
                tc.tile_pool(name="xin", bufs=2) as xinp,
                tc.tile_pool(name="ps1", bufs=1, space="PSUM") as ps1p,
                tc.tile_pool(name="ps2", bufs=2, space="PSUM") as ps2p,
                tc.tile_pool(name="act", bufs=2) as actp,
                tc.tile_pool(name="sbu", bufs=1) as sbup,
                tc.tile_pool(name="cmb", bufs=2) as cmbp,
                tc.tile_pool(name="i1r", bufs=3) as i1rp,
                tc.tile_pool(name="i1ps", bufs=2, space="PSUM") as i1ps,
                tc.tile_pool(name="us", bufs=2) as usp,
            ):
                def emit_i1_group(cg):
                    us = usp.tile([128, R * 64], BF16, name="us")  # (k0, c64)
                    us3 = us[:].rearrange("p (k c) -> p k c", c=64)
                    for hf in range(2):  # c sub-groups of 32
                        c0 = cg * 64 + hf * 32
                        rt = i1rp.tile([66, 2048], BF16, name="rt")
                        d1 = nc.sync.dma_start(
                            rt[0:33, :].rearrange("p (c k0) -> p c k0", c=32),
                            ub4[0, :, c0:c0 + 32, :])
                        d2 = nc.sync.dma_start(
                            rt[33:66, :].rearrange("p (c k0) -> p c k0", c=32),
                            ub4[1, :, c0:c0 + 32, :])
                        kn = (64 * cg + 63) // 96
                        add_dep_helper(d1.ins, funnels[kn].ins, sync=True,
                                       reason="i1-after-combine")
                        add_dep_helper(d2.ins, funnels[kn].ins, sync=True,
                                       reason="i1-after-combine")
                        for j in range(4):
                            s = hf * 4 + j
                            pst = i1ps.tile([128, 512], F32, name="i1p")
                            nc.tensor.matmul(pst[:], vsb[:],
                                             rt[:, j * 512:(j + 1) * 512],
                                             start=True, stop=True)
                            ps3 = pst[:].rearrange("p (c k) -> p k c", c=8)
                            if s % 2 == 0:
                                nc.vector.tensor_copy(
                                    us3[:, :, s * 8:(s + 1) * 8], ps3)
                            else:
                                nc.scalar.copy(
                                    us3[:, :, s * 8:(s + 1) * 8], ps3)
                    nc.sync.dma_start(u1buf[:, :, cg * 64:(cg + 1) * 64], us3)

                for k in range(NBLK):
                    ubuf_dmas = []
                    xr = xinp.tile([BS, F], F32, name="xr")
                    xi = xinp.tile([BS, F], F32, name="xi")
                    nc.sync.dma_start(xr[:], xfreT[k * BS:(k + 1) * BS, :])
                    nc.sync.dma_start(xi[:], xfimT[k * BS:(k + 1) * BS, :])
                    xrb = xinp.tile([BS, F], BF16, name="xrb")
                    xib = xinp.tile([BS, F], BF16, name="xib")
                    nc.gpsimd.tensor_copy(xrb[:], xr[:])
                    nc.gpsimd.tensor_copy(xib[:], xi[:])
                    ksl = slice(k * BS, (k + 1) * BS)
                    sr = sbup.tile([BS, F], F32, name="sr")
                    si = sbup.tile([BS, F], F32, name="si")
                    for o, w in MMCH:
                        p1r = ps1p.tile([BS, 512], F32, name="p1r")
                        nc.tensor.matmul(p1r[:, 0:w], wsb["w1r"][:, ksl],
                                         xrb[:, o:o + w], start=True, stop=False)
                        nc.tensor.matmul(p1r[:, 0:w], wsb["w1in"][:, ksl],
                                         xib[:, o:o + w], start=False, stop=True)
                        o1r = actp.tile([BS, 512], BF16, name="o1r")
                        nc.scalar.activation(o1r[:, 0:w], p1r[:, 0:w], ACTF.Relu,
                                             bias=bsb[:, 0 * NBLK + k:0 * NBLK + k + 1])
                        p1i = ps1p.tile([BS, 512], F32, name="p1i")
                        nc.tensor.matmul(p1i[:, 0:w], wsb["w1r"][:, ksl],
                                         xib[:, o:o + w], start=True, stop=False)
                        nc.tensor.matmul(p1i[:, 0:w], wsb["w1i"][:, ksl],
                                         xrb[:, o:o + w], start=False, stop=True)
                        o1i = actp.tile([BS, 512], BF16, name="o1i")
                        nc.scalar.activation(o1i[:, 0:w], p1i[:, 0:w], ACTF.Relu,
                                             bias=bsb[:, 1 * NBLK + k:1 * NBLK + k + 1])
                        p2r = ps2p.tile([BS, 512], F32, name="p2r")
                        nc.tensor.matmul(p2r[:, 0:w], wsb["w2r"][:, ksl],
                                         o1r[:, 0:w], start=True, stop=False)
                        nc.tensor.matmul(p2r[:, 0:w], wsb["w2in"][:, ksl],
                                         o1i[:, 0:w], start=False, stop=True)
                        p2i = ps2p.tile([BS, 512], F32, name="p2i")
                        nc.tensor.matmul(p2i[:, 0:w], wsb["w2r"][:, ksl],
                                         o1i[:, 0:w], start=True, stop=False)
                        nc.tensor.matmul(p2i[:, 0:w], wsb["w2i"][:, ksl],
                                         o1r[:, 0:w], start=False, stop=True)
                        # softshrink(v + b2) = relu(v + b2 - l) - relu(-v - b2 - l)
                        a1 = actp.tile([BS, 512], F32, name="a1")
                        nc.scalar.activation(a1[:, 0:w], p2r[:, 0:w], ACTF.Relu,
                                             bias=bsb[:, 2 * NBLK + k:2 * NBLK + k + 1])
                        a2 = actp.tile([BS, 512], F32, name="a2")
                        nc.scalar.activation(a2[:, 0:w], p2r[:, 0:w], ACTF.Relu,
                                             scale=-1.0,
                                             bias=bsb[:, 3 * NBLK + k:3 * NBLK + k + 1])
                        nc.vector.tensor_sub(sr[:, o:o + w], a1[:, 0:w], a2[:, 0:w])
                        a3 = actp.tile([BS, 512], F32, name="a3")
                        nc.scalar.activation(a3[:, 0:w], p2i[:, 0:w], ACTF.Relu,
                                             bias=bsb[:, 4 * NBLK + k:4 * NBLK + k + 1])
                        a4 = actp.tile([BS, 512], F32, name="a4")
                        nc.scalar.activation(a4[:, 0:w], p2i[:, 0:w], ACTF.Relu,
                                             scale=-1.0,
                                             bias=bsb[:, 5 * NBLK + k:5 * NBLK + k + 1])
                        nc.vector.tensor_sub(si[:, o:o + w], a3[:, 0:w], a4[:, 0:w])

                    # ---- combine: t = s * xf^2 ; u = t*(w + wh*m) * adj ----
                    for fo, fw in [(0, 1056), (1056, F - 1056)]:
                        fs = slice(fo, fo + fw)
                        x2r = cmbp.tile([BS, 1056], F32, name="x2r")
                        x2h = cmbp.tile([BS, 1056], F32, name="x2h")
                        tmp = cmbp.tile([BS, 1056], F32, name="tmp")
                        x2r_, x2h_, tmp_ = x2r[:, 0:fw], x2h[:, 0:fw], tmp[:, 0:fw]
                        nc.scalar.square(x2r_, xr[:, fs])
                        nc.scalar.square(tmp_, xi[:, fs])
                        nc.vector.tensor_sub(x2r_, x2r_, tmp_)
                        nc.gpsimd.tensor_mul(x2h_, xr[:, fs], xi[:, fs])
                        tr = cmbp.tile([BS, 1056], F32, name="tr")
                        ti = cmbp.tile([BS, 1056], F32, name="ti")
                        tr_, ti_ = tr[:, 0:fw], ti[:, 0:fw]
                        nc.vector.tensor_mul(tmp_, si[:, fs], x2h_)
                        nc.vector.tensor_mul(tr_, sr[:, fs], x2r_)
                        nc.vector.scalar_tensor_tensor(tr_, tmp_, -2.0, tr_,
                                                       op0=ALU.mult, op1=ALU.add)
                        nc.vector.tensor_mul(tmp_, sr[:, fs], x2h_)
                        nc.vector.tensor_mul(ti_, si[:, fs], x2r_)
                        nc.vector.scalar_tensor_tensor(ti_, tmp_, 2.0, ti_,
                                                       op0=ALU.mult, op1=ALU.add)
                        # P = tr*wr - ti*wi ; Q = tr*whr - ti*whi
                        P = cmbp.tile([BS, 1056], F32, name="P")
                        Qt = cmbp.tile([BS, 1056], F32, name="Qt")
                        P_, Qt_ = P[:, 0:fw], Qt[:, 0:fw]
                        wv = lambda t: wvb[:, t * NBLK + k:t * NBLK + k + 1]
                        nc.scalar.mul(tmp_, ti_, wv(1))
                        nc.vector.scalar_tensor_tensor(P_, tr_, wv(0), tmp_,
                                                       op0=ALU.mult,
                                                       op1=ALU.subtract)
                        nc.scalar.mul(tmp_, ti_, wv(3))
                        nc.vector.scalar_tensor_tensor(Qt_, tr_, wv(2), tmp_,
                                                       op0=ALU.mult,
                                                       op1=ALU.subtract)
                        ur = cmbp.tile([BS, 1056], BF16, name="urb")
                        ur_ = ur[:, 0:fw]
                        nc.gpsimd.tensor_mul(tmp_, Qt_, mbc[:, fs])
                        nc.vector.tensor_add(ur_, P_, tmp_)
                        if fo == 0:
                            nc.vector.tensor_scalar_mul(ur[:, 0:1], ur[:, 0:1], 0.5)
                        else:
                            nc.vector.tensor_scalar_mul(
                                ur[:, 2048 - fo:2049 - fo],
                                ur[:, 2048 - fo:2049 - fo], 0.5)
                        ubuf_dmas.append(nc.sync.dma_start(ubuf[0, ksl, fs], ur_))
                        # Pi = tr*wi + ti*wr ; Qi = tr*whi + ti*whr
                        nc.gpsimd.tensor_scalar_mul(tmp_, ti_, wv(0))
                        nc.vector.scalar_tensor_tensor(P_, tr_, wv(1), tmp_,
                                                       op0=ALU.mult, op1=ALU.add)
                        nc.gpsimd.tensor_scalar_mul(tmp_, ti_, wv(2))
                        nc.vector.scalar_tensor_tensor(Qt_, tr_, wv(3), tmp_,
                                                       op0=ALU.mult, op1=ALU.add)
                        ui = cmbp.tile([BS, 1056], BF16, name="uib")
                        ui_ = ui[:, 0:fw]
                        nc.gpsimd.tensor_mul(tmp_, Qt_, mbc[:, fs])
                        nc.vector.tensor_add(ui_, P_, tmp_)
                        if fo == 0:
                            nc.vector.memset(ui[:, 0:1], 0.0)
                        else:
                            nc.vector.memset(ui[:, 2048 - fo:2049 - fo], 0.0)
                        ubuf_dmas.append(nc.sync.dma_start(ubuf[1, ksl, fs], ui_))
                    fn = nc.sync.nop()
                    deps = list(ubuf_dmas)
                    if k == 0:
                        deps += zpad_dmas
                    else:
                        deps.append(funnels[k - 1])
                    for d in deps:
                        add_dep_helper(fn.ins, d.ins, sync=True,
                                       reason="block funnel")
                    funnels[k] = fn
                    for cg in range(12):
                        if (64 * cg + 63) // 96 == k:
                            emit_i1_group(cg)

            # ---- stage I2: out rows 64*b1 + b0 ----
            tc.strict_bb_all_engine_barrier()
            u14 = u1buf[:].rearrange("(h b) k c -> h b k c", h=2)
            with (
                tc.tile_pool(name="i2r", bufs=6) as i2rp,
                tc.tile_pool(name="i2ps", bufs=4, space="PSUM") as i2ps,
            ):
                for qp in range(R // 2):
                    ev = i2rp.tile([64, 2 * C], F32, name="ev2")
                    for half in range(2):
                        q = 2 * qp + half
                        rt = i2rp.tile([128, C], BF16, name="rt2")
                        nc.sync.dma_start(rt[0:64, :], u1buf[q, :, :])
                        nc.sync.dma_start(rt[64:128, :], u1buf[64 + q, :, :])
                        pst = i2ps.tile([64, C], F32, name="i2p")
                        lhs = msb[:, q * 64:(q + 1) * 64]
                        nc.tensor.matmul(pst[:, 0:512], lhs, rt[:, 0:512],
                                         start=True, stop=True)
                        nc.tensor.matmul(pst[:, 512:768], lhs, rt[:, 512:768],
                                         start=True, stop=True)
                        nc.scalar.copy(ev[:, half * C:(half + 1) * C], pst[:])
                    nc.sync.dma_start(
                        out[:, 2 * qp:2 * qp + 2, :],
                        ev[:].rearrange("p (h c) -> p h c", h=2))
    nc.compile()
    return nc


# ------------------------------------------------------------------ host glue
def _quantile_linear(a, q):
    # jnp.quantile default method="linear" over flattened array
    a = np.sort(a, axis=None)
    n = a.shape[0]
    pos = q * (n - 1)
    lo = int(np.floor(pos))
    hi = min(lo + 1, n - 1)
    frac = pos - lo
    return a[lo] * (1 - frac) + a[hi] * frac


def _prep_l2(cw, cwh, w1, b1, w2, b2, m_row):
    """Per-core launch-2 input map minus the xf tensors. m_row: (F,) in {0,1,2}."""
    sv_re = np.ones(F, np.float32)
    sv_re[0] = 0.5
    sv_re[2048] = 0.5
    sv_im = np.ones(F, np.float32)
    sv_im[0] = 0.0
    sv_im[2048] = 0.0
    w1r_ = np.ascontiguousarray(np.concatenate([w1[0, k] for k in range(NBLK)], axis=1))
    w1i_ = np.ascontiguousarray(np.concatenate([w1[1, k] for k in range(NBLK)], axis=1))
    w2r_ = np.ascontiguousarray(np.concatenate([w2[0, k] for k in range(NBLK)], axis=1))
    w2i_ = np.ascontiguousarray(np.concatenate([w2[1, k] for k in range(NBLK)], axis=1))
    bvec = np.zeros((BS, 8 * NBLK), np.float32)
    wvec = np.zeros((BS, 4 * NBLK), np.float32)
    for k in range(NBLK):
        bvec[:, 0 * NBLK + k] = b1[0, k]
        bvec[:, 1 * NBLK + k] = b1[1, k]
        bvec[:, 2 * NBLK + k] = b2[0, k] - LAMBD
        bvec[:, 3 * NBLK + k] = -b2[0, k] - LAMBD
        bvec[:, 4 * NBLK + k] = b2[1, k] - LAMBD
        bvec[:, 5 * NBLK + k] = -b2[1, k] - LAMBD
        bvec[:, 6 * NBLK + k] = b2[0, k] + LAMBD
        bvec[:, 7 * NBLK + k] = b2[1, k] + LAMBD
        sl = slice(k * BS, (k + 1) * BS)
        wvec[:, 0 * NBLK + k] = cw[sl, 0]
        wvec[:, 1 * NBLK + k] = cw[sl, 1]
        wvec[:, 2 * NBLK + k] = cwh[sl, 0]
        wvec[:, 3 * NBLK + k] = cwh[sl, 1]
    import ml_dtypes
    bf = lambda a: np.ascontiguousarray(a).astype(ml_dtypes.bfloat16)
    return {
        "mvs": np.asarray(m_row, np.float32).reshape(1, F),
        "w1r": bf(w1r_), "w1i": bf(w1i_), "w1in": bf(-w1i_),
        "w2r": bf(w2r_), "w2i": bf(w2i_), "w2in": bf(-w2i_),
        "bvec": bvec, "wvec": wvec,
    }


def _masks(en, thr):
    """en: (B,F) energies; returns m (B,F) in {0,1,2}."""
    med = np.sort(en, axis=1)[:, (F - 1) // 2][:, None]  # method="lower"
    nrg = (en / (med + 1e-6)).astype(np.float32)
    thr_high = _quantile_linear(nrg, thr)
    thr_low = _quantile_linear(nrg, LOW_Q)
    mask_high = (nrg > thr_high).astype(np.float32)
    mask_low = (nrg <= thr_low).astype(np.float32)
    return mask_high * (1.0 + mask_low)


def kernel(x, complex_weight, complex_weight_high, w1, b1, w2, b2,
           threshold_param):
    x = np.asarray(x, np.float32)
    cw = np.asarray(complex_weight, np.float32)
    cwh = np.asarray(complex_weight_high, np.float32)
    w1 = np.asarray(w1, np.float32)
    b1 = np.asarray(b1, np.float32)
    w2 = np.asarray(w2, np.float32)
    b2 = np.asarray(b2, np.float32)
    thr = float(np.asarray(threshold_param).reshape(-1)[0])

    if "l1" not in _CACHE:
        _CACHE["l1"] = _build_l1()
    in_maps1 = [{"x": np.ascontiguousarray(x[i])} for i in range(B)]
    _t0 = _time.time()
    res1 = run_bass_kernel_spmd(_CACHE["l1"], in_maps1, core_ids=list(range(B)),
                                trace=TRACE)
    _t1 = _time.time()
    r1 = res1.results if hasattr(res1, "results") else res1

    xf_re = np.stack([np.asarray(r["xf_re"]).reshape(33 * 64, C)[:F] for r in r1])
    xf_im = np.stack([np.asarray(r["xf_im"]).reshape(33 * 64, C)[:F] for r in r1])
    en = np.stack([(np.asarray(r["energy"])[:64] + np.asarray(r["energy"])[64:])
                   .reshape(-1)[:F] for r in r1])
    m = _masks(en, thr)

    if "l2" not in _CACHE:
        _CACHE["l2"] = _build_l2()
    base = _prep_l2(cw, cwh, w1, b1, w2, b2, m[0])
    in_maps2 = []
    for i in range(B):
        im = dict(_prep_l2(cw, cwh, w1, b1, w2, b2, m[i]))
        im["xfreT"] = np.ascontiguousarray(xf_re[i].T)
        im["xfimT"] = np.ascontiguousarray(xf_im[i].T)
        in_maps2.append(im)
    _t2 = _time.time()
    res2 = run_bass_kernel_spmd(_CACHE["l2"], in_maps2, core_ids=list(range(B)),
                                trace=TRACE)
    _t3 = _time.time()
    r2 = res2.results if hasattr(res2, "results") else res2

    out = np.stack([np.asarray(r["out"]).reshape(N, C) for r in r2])
    LAST_NS.clear()
    for res in (res1, res2):
        LAST_NS.append(getattr(res, "exec_time_ns", None))
    LAST_NS.append(("wall_l1_s", _t1 - _t0))
    LAST_NS.append(("wall_l2_s", _t3 - _t2))
    return out.astype(np.float32)



# revision 10
# speedup vs baseline: 1.1391x; 1.1391x over previous
"""Adaptive Spectral Block on 8 Trainium2 NeuronCores.

Strategy: data-parallel over batch (1 sample/core). Two device launches:
  L1: four-step radix-64 forward rfft (fp32 matmuls) + per-frequency energy
  host: quantile thresholds (tiny: 8x2049 values) -> mask scale vectors
  L2: block-diag complex MLP + softshrink + spectral combine + four-step irfft
The mid-FFT transpose is routed through DRAM scratch with large affine DMAs.
"""

import math
import time as _time
import numpy as np

import concourse.bass as bass
import concourse.tile as tile
from concourse import bacc, mybir
from concourse.bass_utils import run_bass_kernel_spmd

F32 = mybir.dt.float32
BF16 = mybir.dt.bfloat16
FR = mybir.dt.float32r
AX = mybir.AxisListType
ALU = mybir.AluOpType
ACTF = mybir.ActivationFunctionType

B, N, C = 8, 4096, 768
R = 64            # radix
F = N // 2 + 1    # 2049
K1Q = 33          # inverse stage-1 contraction length (2112 = 33*64 padded spectrum)
FP = 2112         # padded spectrum length
NBLK, BS = 8, 96  # MLP blocks
LAMBD = 0.01
LOW_Q = 0.5

_CACHE = {}
TRACE = False
LAST_NS = []


# ------------------------------------------------------------------ matrices
def _fwd_mats():
    """Hermitian-deduped forward-FFT matrices.

    Stage A: 64-pt real DFT over n1 -> 64 independent rows
      [Re k1=0..32 (33) | Im k1=1..31 (31)], scaled 1/64.
    Stage B: per q (=k1 of final index f=64*k2+q) twiddled 64-pt DFT over n2
      producing 66 rows [Re k2=0..32 | Im k2=0..32].
    """
    n1 = np.arange(R)
    k1 = np.arange(33)
    DC = np.cos(2 * np.pi * np.outer(n1, k1) / R) / 64.0          # (64, 33)
    k1i = np.arange(1, 32)
    DS = -np.sin(2 * np.pi * np.outer(n1, k1i) / R) / 64.0        # (64, 31)
    dstack = np.concatenate([DC, DS], axis=1).astype(np.float32)  # (64, 64)

    k2 = np.arange(33)
    n2 = np.arange(R)
    tm = np.zeros((R, 128, 66), np.float32)
    for q in range(R):
        ang = 2 * np.pi * (np.outer(k2, n2 * 64) + n2[None, :] * q) / N
        TR, TI = np.cos(ang), -np.sin(ang)   # (33, 64) each
        # zt rows 0:64 hold Re Y(q) over n2; rows 64:128 hold stored Im row
        # (= Im Y(q) for q<=31, = -Im Y(q) for q>=33).
        sgn = 1.0 if q <= 32 else -1.0
        tm[q, :64, :33] = TR.T               # Re out
        tm[q, 64:, :33] = sgn * (-TI.T)
        tm[q, :64, 33:] = TI.T               # Im out
        tm[q, 64:, 33:] = sgn * TR.T
    # pre-arranged for SBUF (p, (q, m)) layout
    tmats = np.ascontiguousarray(tm.transpose(1, 0, 2)).reshape(128, R * 66)
    return dstack, tmats


def _inv_mats():
    b0 = np.arange(R)
    k1q = np.arange(K1Q)
    VC = np.cos(2 * np.pi * np.outer(k1q, b0) / R)
    VS = np.sin(2 * np.pi * np.outer(k1q, b0) / R)
    vstack = np.zeros((66, 128), np.float32)
    vstack[:33, :64] = VC
    vstack[33:, :64] = -VS
    vstack[:33, 64:] = VS
    vstack[33:, 64:] = VC
    vstack *= 2.0 / 64.0

    b1 = np.arange(R)
    k0 = np.arange(R)
    mm = np.zeros((R, 128, 64), np.float32)
    for q in range(R):
        ang = 2 * np.pi * (np.outer(b1, k0 * 64) + k0[None, :] * q) / N
        mm[q, :64] = np.cos(ang).T
        mm[q, 64:] = -np.sin(ang).T
    mmats = np.ascontiguousarray(mm.transpose(1, 0, 2)).reshape(128, R * 64)
    return vstack, mmats


# ------------------------------------------------------------------ launch 1
def _sb_batches():
    """Stage-B batches: (q_list, re_src_row, im_src_row_or_None, out, col0).

    out is "a" (xf_a, col=q) or "b" (xf_b, col=j with q=63-j). Sources are
    ascending ya2 rows so every DMA access pattern stays affine.
    """
    batches = []
    for q0 in range(0, 32, 4):
        im0 = 32 + q0 if q0 > 0 else None  # q0=0 handled specially (q=1..3)
        batches.append((list(range(q0, q0 + 4)), q0, im0, "a", q0))
    batches.append(([32], 32, None, "a", 32))
    for j0 in range(0, 31, 4):
        nb = min(4, 31 - j0)
        qs = [63 - (j0 + t) for t in range(nb)]
        batches.append((qs, 1 + j0, 33 + j0, "b", j0))
    return batches


def _build_l1():
    dstack_np, tmats_np = _fwd_mats()
    nc = bacc.Bacc(None, target_bir_lowering=False)
    x = nc.dram_tensor("x", [N, C], F32, kind="ExternalInput")
    # rows 0:33 = Re k2, rows 33:66 = Im k2; "a" cols = q 0..32,
    # "b" cols = j 0..30 with q = 63 - j
    xf_a = nc.dram_tensor("xf_a", [66, 33, C], BF16, kind="ExternalOutput")
    xf_b = nc.dram_tensor("xf_b", [66, 31, C], BF16, kind="ExternalOutput")
    energy = nc.dram_tensor("energy", [66, 64], F32, kind="ExternalOutput")
    dstack_h = nc.inline_tensor(dstack_np, name="dstack")
    tmats_h = nc.inline_tensor(tmats_np, name="tmats")

    with tile.TileContext(nc) as tc:
        with (
            tc.tile_pool(name="const", bufs=1) as constp,
            tc.tile_pool(name="en", bufs=1) as enp,
            tc.tile_pool(name="dram", bufs=1, space="DRAM") as dramp,
        ):
            dsb = constp.tile([64, 64], F32)
            nc.sync.dma_start(dsb[:], dstack_h[:])
            tsb = constp.tile([128, R * 66], F32)
            nc.sync.dma_start(tsb[:], tmats_h[:])
            en_acc = enp.tile([66, 64], F32)

            # deduped stage-A output: rows [Re k1 0..32 | Im k1 1..31]
            ya = dramp.tile([64, 64, C], F32)  # [k1row, n2, c]

            x3 = x[:].rearrange("(a b) c -> a b c", b=R)  # (n1, n2, c)
            with (
                tc.tile_pool(name="xin", bufs=3) as xp,
                tc.tile_pool(name="ysb", bufs=4) as yp,
                tc.tile_pool(name="psA", bufs=3, space="PSUM") as psA,
            ):
                for nb in range(0, R, 4):
                    xt = xp.tile([64, 4 * C], F32, name="xt")
                    nc.sync.dma_start(
                        xt[:].rearrange("p (j c) -> p j c", j=4),
                        x3[:, nb:nb + 4, :])
                    for h in range(2):  # 2 n2 per psum tile
                        ps = psA.tile([128, C], F32, name="psA")
                        for j2 in range(2):
                            co = (2 * h + j2) * C
                            nc.tensor.matmul(
                                ps[j2 * 64:(j2 + 1) * 64, 0:512], dsb[:],
                                xt[:, co:co + 512], start=True, stop=True)
                            nc.tensor.matmul(
                                ps[j2 * 64:(j2 + 1) * 64, 512:768], dsb[:],
                                xt[:, co + 512:co + 768], start=True, stop=True)
                        ysb = yp.tile([128, C], F32, name="ysb")
                        if h == 0:
                            nc.vector.tensor_copy(ysb[:], ps[:])
                        else:
                            nc.scalar.copy(ysb[:], ps[:])
                        n2 = nb + 2 * h
                        nc.gpsimd.dma_start(
                            ya[:, n2:n2 + 2, :].rearrange("p j c -> j p c"),
                            ysb[:])

            tc.strict_bb_all_engine_barrier()
            with (
                tc.tile_pool(name="zt", bufs=3) as ztp,
                tc.tile_pool(name="psB", bufs=4, space="PSUM") as psB,
                tc.tile_pool(name="sq", bufs=2) as sqp,
                tc.tile_pool(name="evB", bufs=2) as evp,
            ):
                for bi, (qs, re0, im0, dst, col0) in enumerate(_sb_batches()):
                    nb = len(qs)
                    zt = ztp.tile([128, nb * C], F32, name="zt")
                    nc.sync.dma_start(
                        zt[0:64, :].rearrange("p (j c) -> p j c", j=nb),
                        ya[re0:re0 + nb, :, :].rearrange("j p c -> p j c"))
                    if im0 is not None:
                        nc.sync.dma_start(
                            zt[64:128, :].rearrange("p (j c) -> p j c", j=nb),
                            ya[im0:im0 + nb, :, :].rearrange("j p c -> p j c"))
                    elif dst == "a" and col0 == 0:
                        # batch (0..3): Im rows exist for q=1..3 only
                        nc.sync.dma_start(
                            zt[64:128, C:4 * C].rearrange(
                                "p (j c) -> p j c", j=3),
                            ya[33:36, :, :].rearrange("j p c -> p j c"))
                    ev = evp.tile([66, nb * C], BF16, name="evB")
                    for jj, q in enumerate(qs):
                        has_im = not (q == 0 or q == 32)
                        kp = 128 if has_im else 64
                        ps = psB.tile([66, C], F32, name="psB")
                        lhs = tsb[0:kp, q * 66:(q + 1) * 66]
                        co = jj * C
                        nc.tensor.matmul(ps[:, 0:512], lhs,
                                         zt[0:kp, co:co + 512],
                                         start=True, stop=True)
                        nc.tensor.matmul(ps[:, 512:768], lhs,
                                         zt[0:kp, co + 512:co + 768],
                                         start=True, stop=True)
                        sq = sqp.tile([66, C], BF16, name="sq")
                        nc.scalar.activation(sq[:], ps[:], ACTF.Square,
                                             accum_out=en_acc[:, q:q + 1])
                        nc.vector.tensor_copy(ev[:, co:co + C], ps[:])
                    tgt = xf_a if dst == "a" else xf_b
                    nc.sync.dma_start(
                        tgt[:, col0:col0 + nb, :],
                        ev[:].rearrange("p (j c) -> p j c", j=nb))

            nc.sync.dma_start(energy[:], en_acc[:])
    nc.compile()
    return nc


# ------------------------------------------------------------------ launch 2
def _build_l2():
    import ml_dtypes
    vstack_np, mmats_np = _inv_mats()
    vstack_np_bf16 = vstack_np.astype(ml_dtypes.bfloat16)
    mmats_np_bf16 = mmats_np.astype(ml_dtypes.bfloat16)
    nc = bacc.Bacc(None, target_bir_lowering=False)
    xfreT = nc.dram_tensor("xfreT", [C, F], F32, kind="ExternalInput")
    xfimT = nc.dram_tensor("xfimT", [C, F], F32, kind="ExternalInput")
    # m-scale vectors (per-frequency): re: u_r = P*mv1 + Q*mv2 ; im: u_i = Pi*mv3 + Qi*mv4
    mvs = nc.dram_tensor("mvs", [1, F], F32, kind="ExternalInput")
    # weights pre-arranged: (96, 8*96) blocks of lhsT
    w1r = nc.dram_tensor("w1r", [BS, NBLK * BS], BF16, kind="ExternalInput")
    w1i = nc.dram_tensor("w1i", [BS, NBLK * BS], BF16, kind="ExternalInput")
    w1in = nc.dram_tensor("w1in", [BS, NBLK * BS], BF16, kind="ExternalInput")
    w2r = nc.dram_tensor("w2r", [BS, NBLK * BS], BF16, kind="ExternalInput")
    w2i = nc.dram_tensor("w2i", [BS, NBLK * BS], BF16, kind="ExternalInput")
    w2in = nc.dram_tensor("w2in", [BS, NBLK * BS], BF16, kind="ExternalInput")
    # biases / vectors packed (96, 8): col k
    bvec = nc.dram_tensor("bvec", [BS, 8 * NBLK], F32, kind="ExternalInput")
    # bvec cols: [b1r, b1i, b2rm, b2rn, b2im, b2in, _, _] interleaved per k? ->
    # layout: bvec[:, t*NBLK + k] for t in 0..7
    wvec = nc.dram_tensor("wvec", [BS, 4 * NBLK], F32, kind="ExternalInput")
    # wvec cols: [wr, wi, whr, whi] x k
    out = nc.dram_tensor("out", [64, 64, C], F32, kind="ExternalOutput")
    vstack_h = nc.inline_tensor(vstack_np.astype(np.dtype("bfloat16") if False else None) if False else vstack_np_bf16, name="vstack")
    mmats_h = nc.inline_tensor(mmats_np_bf16, name="mmats")

    MMCH = [(0, 512), (512, 512), (1024, 512), (1536, 512), (2048, 1)]

    with tile.TileContext(nc) as tc:
        with (
            tc.tile_pool(name="const", bufs=1) as constp,
            tc.tile_pool(name="dram", bufs=1, space="DRAM") as dramp,
        ):
            wsb = {}
            for nm, h in [("w1r", w1r), ("w1i", w1i), ("w1in", w1in),
                          ("w2r", w2r), ("w2i", w2i), ("w2in", w2in)]:
                t = constp.tile([BS, NBLK * BS], BF16, name=nm)
                nc.sync.dma_start(t[:], h[:])
                wsb[nm] = t
            bsb = constp.tile([BS, 8 * NBLK], F32)
            nc.sync.dma_start(bsb[:], bvec[:])
            wvb = constp.tile([BS, 4 * NBLK], F32)
            nc.sync.dma_start(wvb[:], wvec[:])
            vsb = constp.tile([66, 128], BF16)
            nc.sync.dma_start(vsb[:], vstack_h[:])
            msb = constp.tile([128, R * 64], BF16)
            nc.sync.dma_start(msb[:], mmats_h[:])

            ubuf = dramp.tile([2, C, FP], BF16)   # [re/im, c, f]
            u1buf = dramp.tile([128, R, C], BF16)  # [b0stack, k0, c]

            # ---- m-vector broadcast to (96, F) via K=1 matmul ----
            onesb = constp.tile([1, BS], F32)
            nc.vector.memset(onesb[:], 1.0)
            mbc = constp.tile([BS, F], BF16, name="mbc")
            with (
                tc.tile_pool(name="mvstage", bufs=1) as mvp,
                tc.tile_pool(name="psm", bufs=4, space="PSUM") as psm,
            ):
                mvsb = mvp.tile([1, F], F32, name="mvs")
                nc.sync.dma_start(mvsb[:], mvs[0, :])
                for o, w in [(0, 512), (512, 512), (1024, 512),
                             (1536, 512), (2048, 1)]:
                    pst = psm.tile([BS, 512], F32, name="psb")
                    nc.tensor.matmul(pst[:, 0:w], onesb[:],
                                     mvsb[:, o:o + w],
                                     start=True, stop=True)
                    nc.scalar.copy(mbc[:, o:o + w], pst[:, 0:w])

            # ---- zero the spectrum pad f in [2049, 2112) ----
            zpad = constp.tile([128, FP - F], BF16)
            nc.vector.memset(zpad[:], 0.0)
            ub2 = ubuf[:].rearrange("h c f -> (h c) f")
            zpad_dmas = []
            for j in range(2 * C // 128):
                zpad_dmas.append(
                    nc.sync.dma_start(ub2[j * 128:(j + 1) * 128, F:FP], zpad[:]))
            from concourse.tile import add_dep_helper
            ub4 = ubuf[:].rearrange("h c (k1 k0) -> h k1 c k0", k0=R)
            funnels = {}

            # ---- per-block MLP + combine ----
            with (
                tc.tile_pool(name="xin", bufs=2) as xinp,
                tc.tile_pool(name="ps1", bufs=1, space="PSUM") as ps1p,
                tc.tile_pool(name="ps2", bufs=2, space="PSUM") as ps2p,
                tc.tile_pool(name="act", bufs=2) as actp,
                tc.tile_pool(name="sbu", bufs=1) as sbup,
                tc.tile_pool(name="cmb", bufs=2) as cmbp,
                tc.tile_pool(name="i1r", bufs=3) as i1rp,
                tc.tile_pool(name="i1ps", bufs=2, space="PSUM") as i1ps,
                tc.tile_pool(name="us", bufs=2) as usp,
            ):
                def emit_i1_group(cg):
                    us = usp.tile([128, R * 64], BF16, name="us")  # (k0, c64)
                    us3 = us[:].rearrange("p (k c) -> p k c", c=64)
                    for hf in range(2):  # c sub-groups of 32
                        c0 = cg * 64 + hf * 32
                        rt = i1rp.tile([66, 2048], BF16, name="rt")
                        d1 = nc.sync.dma_start(
                            rt[0:33, :].rearrange("p (c k0) -> p c k0", c=32),
                            ub4[0, :, c0:c0 + 32, :])
                        d2 = nc.sync.dma_start(
                            rt[33:66, :].rearrange("p (c k0) -> p c k0", c=32),
                            ub4[1, :, c0:c0 + 32, :])
                        kn = (64 * cg + 63) // 96
                        add_dep_helper(d1.ins, funnels[kn].ins, sync=True,
                                       reason="i1-after-combine")
                        add_dep_helper(d2.ins, funnels[kn].ins, sync=True,
                                       reason="i1-after-combine")
                        for j in range(4):
                            s = hf * 4 + j
                            pst = i1ps.tile([128, 512], F32, name="i1p")
                            nc.tensor.matmul(pst[:], vsb[:],
                                             rt[:, j * 512:(j + 1) * 512],
                                             start=True, stop=True)
                            ps3 = pst[:].rearrange("p (c k) -> p k c", c=8)
                            if s % 2 == 0:
                                nc.vector.tensor_copy(
                                    us3[:, :, s * 8:(s + 1) * 8], ps3)
                            else:
                                nc.scalar.copy(
                                    us3[:, :, s * 8:(s + 1) * 8], ps3)
                    nc.sync.dma_start(u1buf[:, :, cg * 64:(cg + 1) * 64], us3)

                for k in range(NBLK):
                    ubuf_dmas = []
                    xr = xinp.tile([BS, F], F32, name="xr")
                    xi = xinp.tile([BS, F], F32, name="xi")
                    nc.sync.dma_start(xr[:], xfreT[k * BS:(k + 1) * BS, :])
                    nc.sync.dma_start(xi[:], xfimT[k * BS:(k + 1) * BS, :])
                    xrb = xinp.tile([BS, F], BF16, name="xrb")
                    xib = xinp.tile([BS, F], BF16, name="xib")
                    nc.gpsimd.tensor_copy(xrb[:], xr[:])
                    nc.gpsimd.tensor_copy(xib[:], xi[:])
                    ksl = slice(k * BS, (k + 1) * BS)
                    sr = sbup.tile([BS, F], F32, name="sr")
                    si = sbup.tile([BS, F], F32, name="si")
                    for o, w in MMCH:
                        p1r = ps1p.tile([BS, 512], F32, name="p1r")
                        nc.tensor.matmul(p1r[:, 0:w], wsb["w1r"][:, ksl],
                                         xrb[:, o:o + w], start=True, stop=False)
                        nc.tensor.matmul(p1r[:, 0:w], wsb["w1in"][:, ksl],
                                         xib[:, o:o + w], start=False, stop=True)
                        o1r = actp.tile([BS, 512], BF16, name="o1r")
                        nc.scalar.activation(o1r[:, 0:w], p1r[:, 0:w], ACTF.Relu,
                                             bias=bsb[:, 0 * NBLK + k:0 * NBLK + k + 1])
                        p1i = ps1p.tile([BS, 512], F32, name="p1i")
                        nc.tensor.matmul(p1i[:, 0:w], wsb["w1r"][:, ksl],
                                         xib[:, o:o + w], start=True, stop=False)
                        nc.tensor.matmul(p1i[:, 0:w], wsb["w1i"][:, ksl],
                                         xrb[:, o:o + w], start=False, stop=True)
                        o1i = actp.tile([BS, 512], BF16, name="o1i")
                        nc.scalar.activation(o1i[:, 0:w], p1i[:, 0:w], ACTF.Relu,
                                             bias=bsb[:, 1 * NBLK + k:1 * NBLK + k + 1])
                        p2r = ps2p.tile([BS, 512], F32, name="p2r")
                        nc.tensor.matmul(p2r[:, 0:w], wsb["w2r"][:, ksl],
                                         o1r[:, 0:w], start=True, stop=False)
                        nc.tensor.matmul(p2r[:, 0:w], wsb["w2in"][:, ksl],
                                         o1i[:, 0:w], start=False, stop=True)
                        p2i = ps2p.tile([BS, 512], F32, name="p2i")
                        nc.tensor.matmul(p2i[:, 0:w], wsb["w2r"][:, ksl],
                                         o1i[:, 0:w], start=True, stop=False)
                        nc.tensor.matmul(p2i[:, 0:w], wsb["w2i"][:, ksl],
                                         o1r[:, 0:w], start=False, stop=True)
                        # softshrink(v + b2) = relu(v + b2 - l) - relu(-v - b2 - l)
                        a1 = actp.tile([BS, 512], F32, name="a1")
                        nc.scalar.activation(a1[:, 0:w], p2r[:, 0:w], ACTF.Relu,
                                             bias=bsb[:, 2 * NBLK + k:2 * NBLK + k + 1])
                        a2 = actp.tile([BS, 512], F32, name="a2")
                        nc.scalar.activation(a2[:, 0:w], p2r[:, 0:w], ACTF.Relu,
                                             scale=-1.0,
                                             bias=bsb[:, 3 * NBLK + k:3 * NBLK + k + 1])
                        nc.vector.tensor_sub(sr[:, o:o + w], a1[:, 0:w], a2[:, 0:w])
                        a3 = actp.tile([BS, 512], F32, name="a3")
                        nc.scalar.activation(a3[:, 0:w], p2i[:, 0:w], ACTF.Relu,
                                             bias=bsb[:, 4 * NBLK + k:4 * NBLK + k + 1])
                        a4 = actp.tile([BS, 512], F32, name="a4")
                        nc.scalar.activation(a4[:, 0:w], p2i[:, 0:w], ACTF.Relu,
                                             scale=-1.0,
                                             bias=bsb[:, 5 * NBLK + k:5 * NBLK + k + 1])
                        nc.vector.tensor_sub(si[:, o:o + w], a3[:, 0:w], a4[:, 0:w])

                    # ---- combine: t = s * xf^2 ; u = t*(w + wh*m) * adj ----
                    for fo, fw in [(0, 1056), (1056, F - 1056)]:
                        fs = slice(fo, fo + fw)
                        x2r = cmbp.tile([BS, 1056], F32, name="x2r")
                        x2h = cmbp.tile([BS, 1056], F32, name="x2h")
                        tmp = cmbp.tile([BS, 1056], F32, name="tmp")
                        x2r_, x2h_, tmp_ = x2r[:, 0:fw], x2h[:, 0:fw], tmp[:, 0:fw]
                        nc.scalar.square(x2r_, xr[:, fs])
                        nc.scalar.square(tmp_, xi[:, fs])
                        nc.vector.tensor_sub(x2r_, x2r_, tmp_)
                        nc.gpsimd.tensor_mul(x2h_, xr[:, fs], xi[:, fs])
                        tr = cmbp.tile([BS, 1056], F32, name="tr")
                        ti = cmbp.tile([BS, 1056], F32, name="ti")
                        tr_, ti_ = tr[:, 0:fw], ti[:, 0:fw]
                        nc.vector.tensor_mul(tmp_, si[:, fs], x2h_)
                        nc.vector.tensor_mul(tr_, sr[:, fs], x2r_)
                        nc.vector.scalar_tensor_tensor(tr_, tmp_, -2.0, tr_,
                                                       op0=ALU.mult, op1=ALU.add)
                        nc.vector.tensor_mul(tmp_, sr[:, fs], x2h_)
                        nc.vector.tensor_mul(ti_, si[:, fs], x2r_)
                        nc.vector.scalar_tensor_tensor(ti_, tmp_, 2.0, ti_,
                                                       op0=ALU.mult, op1=ALU.add)
                        # P = tr*wr - ti*wi ; Q = tr*whr - ti*whi
                        P = cmbp.tile([BS, 1056], F32, name="P")
                        Qt = cmbp.tile([BS, 1056], F32, name="Qt")
                        P_, Qt_ = P[:, 0:fw], Qt[:, 0:fw]
                        wv = lambda t: wvb[:, t * NBLK + k:t * NBLK + k + 1]
                        nc.scalar.mul(tmp_, ti_, wv(1))
                        nc.vector.scalar_tensor_tensor(P_, tr_, wv(0), tmp_,
                                                       op0=ALU.mult,
                                                       op1=ALU.subtract)
                        nc.scalar.mul(tmp_, ti_, wv(3))
                        nc.vector.scalar_tensor_tensor(Qt_, tr_, wv(2), tmp_,
                                                       op0=ALU.mult,
                                                       op1=ALU.subtract)
                        ur = cmbp.tile([BS, 1056], BF16, name="urb")
                        ur_ = ur[:, 0:fw]
                        nc.gpsimd.tensor_mul(tmp_, Qt_, mbc[:, fs])
                        nc.vector.tensor_add(ur_, P_, tmp_)
                        if fo == 0:
                            nc.vector.tensor_scalar_mul(ur[:, 0:1], ur[:, 0:1], 0.5)
                        else:
                            nc.vector.tensor_scalar_mul(
                                ur[:, 2048 - fo:2049 - fo],
                                ur[:, 2048 - fo:2049 - fo], 0.5)
                        ubuf_dmas.append(nc.sync.dma_start(ubuf[0, ksl, fs], ur_))
                        # Pi = tr*wi + ti*wr ; Qi = tr*whi + ti*whr
                        nc.gpsimd.tensor_scalar_mul(tmp_, ti_, wv(0))
                        nc.vector.scalar_tensor_tensor(P_, tr_, wv(1), tmp_,
                                                       op0=ALU.mult, op1=ALU.add)
                        nc.gpsimd.tensor_scalar_mul(tmp_, ti_, wv(2))
                        nc.vector.scalar_tensor_tensor(Qt_, tr_, wv(3), tmp_,
                                                       op0=ALU.mult, op1=ALU.add)
                        ui = cmbp.tile([BS, 1056], BF16, name="uib")
                        ui_ = ui[:, 0:fw]
                        nc.gpsimd.tensor_mul(tmp_, Qt_, mbc[:, fs])
                        nc.vector.tensor_add(ui_, P_, tmp_)
                        if fo == 0:
                            nc.vector.memset(ui[:, 0:1], 0.0)
                        else:
                            nc.vector.memset(ui[:, 2048 - fo:2049 - fo], 0.0)
                        ubuf_dmas.append(nc.sync.dma_start(ubuf[1, ksl, fs], ui_))
                    fn = nc.sync.nop()
                    deps = list(ubuf_dmas)
                    if k == 0:
                        deps += zpad_dmas
                    else:
                        deps.append(funnels[k - 1])
                    for d in deps:
                        add_dep_helper(fn.ins, d.ins, sync=True,
                                       reason="block funnel")
                    funnels[k] = fn
                    for cg in range(12):
                        if (64 * cg + 63) // 96 == k:
                            emit_i1_group(cg)

            # ---- stage I2: out rows 64*b1 + b0 ----
            tc.strict_bb_all_engine_barrier()
            u14 = u1buf[:].rearrange("(h b) k c -> h b k c", h=2)
            with (
                tc.tile_pool(name="i2r", bufs=6) as i2rp,
                tc.tile_pool(name="i2ps", bufs=4, space="PSUM") as i2ps,
            ):
                for qp in range(R // 2):
                    ev = i2rp.tile([64, 2 * C], F32, name="ev2")
                    for half in range(2):
                        q = 2 * qp + half
                        rt = i2rp.tile([128, C], BF16, name="rt2")
                        nc.sync.dma_start(rt[0:64, :], u1buf[q, :, :])
                        nc.sync.dma_start(rt[64:128, :], u1buf[64 + q, :, :])
                        pst = i2ps.tile([64, C], F32, name="i2p")
                        lhs = msb[:, q * 64:(q + 1) * 64]
                        nc.tensor.matmul(pst[:, 0:512], lhs, rt[:, 0:512],
                                         start=True, stop=True)
                        nc.tensor.matmul(pst[:, 512:768], lhs, rt[:, 512:768],
                                         start=True, stop=True)
                        nc.scalar.copy(ev[:, half * C:(half + 1) * C], pst[:])
                    nc.sync.dma_start(
                        out[:, 2 * qp:2 * qp + 2, :],
                        ev[:].rearrange("p (h c) -> p h c", h=2))
    nc.compile()
    return nc


# ------------------------------------------------------------------ host glue
def _quantile_linear(a, q):
    # jnp.quantile default method="linear" over flattened array
    a = np.sort(a, axis=None)
    n = a.shape[0]
    pos = q * (n - 1)
    lo = int(np.floor(pos))
    hi = min(lo + 1, n - 1)
    frac = pos - lo
    return a[lo] * (1 - frac) + a[hi] * frac


def _prep_l2(cw, cwh, w1, b1, w2, b2, m_row):
    """Per-core launch-2 input map minus the xf tensors. m_row: (F,) in {0,1,2}."""
    sv_re = np.ones(F, np.float32)
    sv_re[0] = 0.5
    sv_re[2048] = 0.5
    sv_im = np.ones(F, np.float32)
    sv_im[0] = 0.0
    sv_im[2048] = 0.0
    w1r_ = np.ascontiguousarray(np.concatenate([w1[0, k] for k in range(NBLK)], axis=1))
    w1i_ = np.ascontiguousarray(np.concatenate([w1[1, k] for k in range(NBLK)], axis=1))
    w2r_ = np.ascontiguousarray(np.concatenate([w2[0, k] for k in range(NBLK)], axis=1))
    w2i_ = np.ascontiguousarray(np.concatenate([w2[1, k] for k in range(NBLK)], axis=1))
    bvec = np.zeros((BS, 8 * NBLK), np.float32)
    wvec = np.zeros((BS, 4 * NBLK), np.float32)
    for k in range(NBLK):
        bvec[:, 0 * NBLK + k] = b1[0, k]
        bvec[:, 1 * NBLK + k] = b1[1, k]
        bvec[:, 2 * NBLK + k] = b2[0, k] - LAMBD
        bvec[:, 3 * NBLK + k] = -b2[0, k] - LAMBD
        bvec[:, 4 * NBLK + k] = b2[1, k] - LAMBD
        bvec[:, 5 * NBLK + k] = -b2[1, k] - LAMBD
        bvec[:, 6 * NBLK + k] = b2[0, k] + LAMBD
        bvec[:, 7 * NBLK + k] = b2[1, k] + LAMBD
        sl = slice(k * BS, (k + 1) * BS)
        wvec[:, 0 * NBLK + k] = cw[sl, 0]
        wvec[:, 1 * NBLK + k] = cw[sl, 1]
        wvec[:, 2 * NBLK + k] = cwh[sl, 0]
        wvec[:, 3 * NBLK + k] = cwh[sl, 1]
    import ml_dtypes
    bf = lambda a: np.ascontiguousarray(a).astype(ml_dtypes.bfloat16)
    return {
        "mvs": np.asarray(m_row, np.float32).reshape(1, F),
        "w1r": bf(w1r_), "w1i": bf(w1i_), "w1in": bf(-w1i_),
        "w2r": bf(w2r_), "w2i": bf(w2i_), "w2in": bf(-w2i_),
        "bvec": bvec, "wvec": wvec,
    }


def _masks(en, thr):
    """en: (B,F) energies; returns m (B,F) in {0,1,2}."""
    med = np.sort(en, axis=1)[:, (F - 1) // 2][:, None]  # method="lower"
    nrg = (en / (med + 1e-6)).astype(np.float32)
    thr_high = _quantile_linear(nrg, thr)
    thr_low = _quantile_linear(nrg, LOW_Q)
    mask_high = (nrg > thr_high).astype(np.float32)
    mask_low = (nrg <= thr_low).astype(np.float32)
    return mask_high * (1.0 + mask_low)


def kernel(x, complex_weight, complex_weight_high, w1, b1, w2, b2,
           threshold_param):
    x = np.asarray(x, np.float32)
    cw = np.asarray(complex_weight, np.float32)
    cwh = np.asarray(complex_weight_high, np.float32)
    w1 = np.asarray(w1, np.float32)
    b1 = np.asarray(b1, np.float32)
    w2 = np.asarray(w2, np.float32)
    b2 = np.asarray(b2, np.float32)
    thr = float(np.asarray(threshold_param).reshape(-1)[0])

    if "l1" not in _CACHE:
        _CACHE["l1"] = _build_l1()
    in_maps1 = [{"x": np.ascontiguousarray(x[i])} for i in range(B)]
    _t0 = _time.time()
    res1 = run_bass_kernel_spmd(_CACHE["l1"], in_maps1, core_ids=list(range(B)),
                                trace=TRACE)
    _t1 = _time.time()
    r1 = res1.results if hasattr(res1, "results") else res1

    xf_re, xf_im, en = [], [], []
    for r in r1:
        full = np.zeros((66, 64, C), np.float32)
        full[:, 0:33] = np.asarray(r["xf_a"]).astype(np.float32)
        full[:, 33:64] = np.asarray(r["xf_b"]).astype(np.float32)[:, ::-1]
        xf_re.append(full[0:33].reshape(33 * 64, C)[:F])
        xf_im.append(full[33:66].reshape(33 * 64, C)[:F])
        e = np.asarray(r["energy"])
        en.append((e[0:33] + e[33:66]).reshape(-1)[:F])
    xf_re, xf_im, en = np.stack(xf_re), np.stack(xf_im), np.stack(en)
    m = _masks(en, thr)

    if "l2" not in _CACHE:
        _CACHE["l2"] = _build_l2()
    base = _prep_l2(cw, cwh, w1, b1, w2, b2, m[0])
    in_maps2 = []
    for i in range(B):
        im = dict(_prep_l2(cw, cwh, w1, b1, w2, b2, m[i]))
        im["xfreT"] = np.ascontiguousarray(xf_re[i].T)
        im["xfimT"] = np.ascontiguousarray(xf_im[i].T)
        in_maps2.append(im)
    _t2 = _time.time()
    res2 = run_bass_kernel_spmd(_CACHE["l2"], in_maps2, core_ids=list(range(B)),
                                trace=TRACE)
    _t3 = _time.time()
    r2 = res2.results if hasattr(res2, "results") else res2

    out = np.stack([np.asarray(r["out"]).reshape(N, C) for r in r2])
    LAST_NS.clear()
    for res in (res1, res2):
        LAST_NS.append(getattr(res, "exec_time_ns", None))
    LAST_NS.append(("wall_l1_s", _t1 - _t0))
    LAST_NS.append(("wall_l2_s", _t3 - _t2))
    return out.astype(np.float32)



# revision 19
# speedup vs baseline: 1.3226x; 1.1611x over previous
"""Adaptive Spectral Block on 8 Trainium2 NeuronCores.

Strategy: data-parallel over batch (1 sample/core). Two device launches:
  L1: four-step radix-64 forward rfft (fp32 matmuls) + per-frequency energy
  host: quantile thresholds (tiny: 8x2049 values) -> mask scale vectors
  L2: block-diag complex MLP + softshrink + spectral combine + four-step irfft
The mid-FFT transpose is routed through DRAM scratch with large affine DMAs.
"""

import math
import time as _time
import numpy as np

import concourse.bass as bass
import concourse.tile as tile
from concourse import bacc, mybir
from concourse.bass_utils import run_bass_kernel_spmd

F32 = mybir.dt.float32
BF16 = mybir.dt.bfloat16
FR = mybir.dt.float32r
AX = mybir.AxisListType
ALU = mybir.AluOpType
ACTF = mybir.ActivationFunctionType

B, N, C = 8, 4096, 768
R = 64            # radix
F = N // 2 + 1    # 2049
K1Q = 33          # inverse stage-1 contraction length (2112 = 33*64 padded spectrum)
FP = 2112         # padded spectrum length
NBLK, BS = 8, 96  # MLP blocks
LAMBD = 0.01
LOW_Q = 0.5

_CACHE = {}
TRACE = False
LAST_NS = []


# ------------------------------------------------------------------ matrices
def _fwd_mats():
    """Hermitian-deduped forward-FFT matrices.

    Stage A: 64-pt real DFT over n1 -> 64 independent rows
      [Re k1=0..32 (33) | Im k1=1..31 (31)], scaled 1/64.
    Stage B: per q (=k1 of final index f=64*k2+q) twiddled 64-pt DFT over n2
      producing 66 rows [Re k2=0..32 | Im k2=0..32].
    """
    n1 = np.arange(R)
    k1 = np.arange(33)
    DC = np.cos(2 * np.pi * np.outer(n1, k1) / R) / 64.0          # (64, 33)
    k1i = np.arange(1, 32)
    DS = -np.sin(2 * np.pi * np.outer(n1, k1i) / R) / 64.0        # (64, 31)
    dstack = np.concatenate([DC, DS], axis=1).astype(np.float32)  # (64, 64)

    k2 = np.arange(33)
    n2 = np.arange(R)
    tm = np.zeros((R, 128, 66), np.float32)
    for q in range(R):
        ang = 2 * np.pi * (np.outer(k2, n2 * 64) + n2[None, :] * q) / N
        TR, TI = np.cos(ang), -np.sin(ang)   # (33, 64) each
        # zt rows 0:64 hold Re Y(q) over n2; rows 64:128 hold stored Im row
        # (= Im Y(q) for q<=31, = -Im Y(q) for q>=33).
        sgn = 1.0 if q <= 32 else -1.0
        tm[q, :64, :33] = TR.T               # Re out
        tm[q, 64:, :33] = sgn * (-TI.T)
        tm[q, :64, 33:] = TI.T               # Im out
        tm[q, 64:, 33:] = sgn * TR.T
    # pre-arranged for SBUF (p, (q, m)) layout
    tmats = np.ascontiguousarray(tm.transpose(1, 0, 2)).reshape(128, R * 66)
    return dstack, tmats


def _inv_mats():
    b0 = np.arange(R)
    k1q = np.arange(K1Q)
    VC = np.cos(2 * np.pi * np.outer(k1q, b0) / R)
    VS = np.sin(2 * np.pi * np.outer(k1q, b0) / R)
    vstack = np.zeros((66, 128), np.float32)
    vstack[:33, :64] = VC
    vstack[33:, :64] = -VS
    vstack[:33, 64:] = VS
    vstack[33:, 64:] = VC
    vstack *= 2.0 / 64.0

    b1 = np.arange(R)
    k0 = np.arange(R)
    mm = np.zeros((R, 128, 64), np.float32)
    for q in range(R):
        ang = 2 * np.pi * (np.outer(b1, k0 * 64) + k0[None, :] * q) / N
        mm[q, :64] = np.cos(ang).T
        mm[q, 64:] = -np.sin(ang).T
    mmats = np.ascontiguousarray(mm.transpose(1, 0, 2)).reshape(128, R * 64)
    return vstack, mmats


# ------------------------------------------------------------------ launch 1
def _sb_batches():
    """Stage-B batches: (q_list, re_src_row, im_src_row_or_None, out, col0).

    out is "a" (xf_a, col=q) or "b" (xf_b, col=j with q=63-j). Sources are
    ascending ya2 rows so every DMA access pattern stays affine.
    """
    batches = []
    for q0 in range(0, 32, 4):
        im0 = 32 + q0 if q0 > 0 else None  # q0=0 handled specially (q=1..3)
        batches.append((list(range(q0, q0 + 4)), q0, im0, "a", q0))
    batches.append(([32], 32, None, "a", 32))
    for j0 in range(0, 31, 4):
        nb = min(4, 31 - j0)
        qs = [63 - (j0 + t) for t in range(nb)]
        batches.append((qs, 1 + j0, 33 + j0, "b", j0))
    return batches


def _build_l1():
    dstack_np, tmats_np = _fwd_mats()
    nc = bacc.Bacc(None, target_bir_lowering=False)
    x = nc.dram_tensor("x", [N, C], F32, kind="ExternalInput")
    # rows 0:33 = Re k2, rows 33:66 = Im k2; "a" cols = q 0..32,
    # "b" cols = j 0..30 with q = 63 - j
    xf_a = nc.dram_tensor("xf_a", [66, 33, C], BF16, kind="ExternalOutput")
    xf_b = nc.dram_tensor("xf_b", [66, 31, C], BF16, kind="ExternalOutput")
    energy = nc.dram_tensor("energy", [66, 64], F32, kind="ExternalOutput")
    dstack_h = nc.inline_tensor(dstack_np, name="dstack")
    tmats_h = nc.inline_tensor(tmats_np, name="tmats")

    with tile.TileContext(nc) as tc:
        with (
            tc.tile_pool(name="const", bufs=1) as constp,
            tc.tile_pool(name="en", bufs=1) as enp,
            tc.tile_pool(name="dram", bufs=1, space="DRAM") as dramp,
        ):
            dsb = constp.tile([64, 64], F32)
            nc.sync.dma_start(dsb[:], dstack_h[:])
            tsb = constp.tile([128, R * 66], F32)
            nc.sync.dma_start(tsb[:], tmats_h[:])
            en_acc = enp.tile([66, 64], F32)

            # deduped stage-A output: rows [Re k1 0..32 | Im k1 1..31]
            ya = dramp.tile([64, 64, C], F32)  # [k1row, n2, c]

            x3 = x[:].rearrange("(a b) c -> a b c", b=R)  # (n1, n2, c)
            with (
                tc.tile_pool(name="xin", bufs=3) as xp,
                tc.tile_pool(name="ysb", bufs=4) as yp,
                tc.tile_pool(name="psA", bufs=3, space="PSUM") as psA,
            ):
                for nb in range(0, R, 4):
                    xt = xp.tile([64, 4 * C], F32, name="xt")
                    nc.sync.dma_start(
                        xt[:].rearrange("p (j c) -> p j c", j=4),
                        x3[:, nb:nb + 4, :])
                    for h in range(2):  # 2 n2 per psum tile
                        ps = psA.tile([128, C], F32, name="psA")
                        for j2 in range(2):
                            co = (2 * h + j2) * C
                            nc.tensor.matmul(
                                ps[j2 * 64:(j2 + 1) * 64, 0:512], dsb[:],
                                xt[:, co:co + 512], start=True, stop=True)
                            nc.tensor.matmul(
                                ps[j2 * 64:(j2 + 1) * 64, 512:768], dsb[:],
                                xt[:, co + 512:co + 768], start=True, stop=True)
                        ysb = yp.tile([128, C], F32, name="ysb")
                        if h == 0:
                            nc.vector.tensor_copy(ysb[:], ps[:])
                        else:
                            nc.scalar.copy(ysb[:], ps[:])
                        n2 = nb + 2 * h
                        nc.gpsimd.dma_start(
                            ya[:, n2:n2 + 2, :].rearrange("p j c -> j p c"),
                            ysb[:])

            tc.strict_bb_all_engine_barrier()
            with (
                tc.tile_pool(name="zt", bufs=3) as ztp,
                tc.tile_pool(name="psB", bufs=4, space="PSUM") as psB,
                tc.tile_pool(name="sq", bufs=2) as sqp,
                tc.tile_pool(name="evB", bufs=2) as evp,
            ):
                for bi, (qs, re0, im0, dst, col0) in enumerate(_sb_batches()):
                    nb = len(qs)
                    zt = ztp.tile([128, nb * C], F32, name="zt")
                    nc.sync.dma_start(
                        zt[0:64, :].rearrange("p (j c) -> p j c", j=nb),
                        ya[re0:re0 + nb, :, :].rearrange("j p c -> p j c"))
                    if im0 is not None:
                        nc.sync.dma_start(
                            zt[64:128, :].rearrange("p (j c) -> p j c", j=nb),
                            ya[im0:im0 + nb, :, :].rearrange("j p c -> p j c"))
                    elif dst == "a" and col0 == 0:
                        # batch (0..3): Im rows exist for q=1..3 only
                        nc.sync.dma_start(
                            zt[64:128, C:4 * C].rearrange(
                                "p (j c) -> p j c", j=3),
                            ya[33:36, :, :].rearrange("j p c -> p j c"))
                    ev = evp.tile([66, nb * C], BF16, name="evB")
                    for jj, q in enumerate(qs):
                        has_im = not (q == 0 or q == 32)
                        kp = 128 if has_im else 64
                        ps = psB.tile([66, C], F32, name="psB")
                        lhs = tsb[0:kp, q * 66:(q + 1) * 66]
                        co = jj * C
                        nc.tensor.matmul(ps[:, 0:512], lhs,
                                         zt[0:kp, co:co + 512],
                                         start=True, stop=True)
                        nc.tensor.matmul(ps[:, 512:768], lhs,
                                         zt[0:kp, co + 512:co + 768],
                                         start=True, stop=True)
                        sq = sqp.tile([66, C], BF16, name="sq")
                        nc.scalar.activation(sq[:], ps[:], ACTF.Square,
                                             accum_out=en_acc[:, q:q + 1])
                        nc.vector.tensor_copy(ev[:, co:co + C], ps[:])
                    tgt = xf_a if dst == "a" else xf_b
                    nc.sync.dma_start(
                        tgt[:, col0:col0 + nb, :],
                        ev[:].rearrange("p (j c) -> p j c", j=nb))

            nc.sync.dma_start(energy[:], en_acc[:])
    nc.compile()
    return nc


# ------------------------------------------------------------------ launch 2
def _build_l2():
    import ml_dtypes
    vstack_np, mmats_np = _inv_mats()
    vstack_bf = vstack_np.astype(ml_dtypes.bfloat16)
    mmats_bf = mmats_np.astype(ml_dtypes.bfloat16)
    nc = bacc.Bacc(None, target_bir_lowering=False)
    # xfT rows: 0 = Re, 1 = Im; [2, C, F] bf16
    xfT = nc.dram_tensor("xfT", [2, C, F], BF16, kind="ExternalInput")
    mvs = nc.dram_tensor("mvs", [1, F], BF16, kind="ExternalInput")
    w1r = nc.dram_tensor("w1r", [BS, NBLK * BS], BF16, kind="ExternalInput")
    w1i = nc.dram_tensor("w1i", [BS, NBLK * BS], BF16, kind="ExternalInput")
    w1in = nc.dram_tensor("w1in", [BS, NBLK * BS], BF16, kind="ExternalInput")
    w2r = nc.dram_tensor("w2r", [BS, NBLK * BS], BF16, kind="ExternalInput")
    w2i = nc.dram_tensor("w2i", [BS, NBLK * BS], BF16, kind="ExternalInput")
    w2in = nc.dram_tensor("w2in", [BS, NBLK * BS], BF16, kind="ExternalInput")
    bvec = nc.dram_tensor("bvec", [BS, 8 * NBLK], F32, kind="ExternalInput")
    wvec = nc.dram_tensor("wvec", [BS, 4 * NBLK], F32, kind="ExternalInput")
    outb = nc.dram_tensor("outb", [64, 64, C], BF16, kind="ExternalOutput")
    vstack_h = nc.inline_tensor(vstack_bf, name="vstack")
    mmats_h = nc.inline_tensor(mmats_bf, name="mmats")

    from concourse.tile import add_dep_helper
    FCH = [(0, 1056), (1056, F - 1056)]          # combine chunks (sbuf bf16)
    MCH = [(0, 1024), (1024, 1024), (2048, 1)]   # matmul/psum chunks


    def _mm(ncobj, pst, lhs, rhs_ap, o, w, start, stop):
        # matmul free dim is limited to 512 fp32 columns; split wide chunks
        for so in range(0, w, 512):
            sw = min(512, w - so)
            ncobj.tensor.matmul(pst[:, so:so + sw], lhs,
                                rhs_ap[:, o + so:o + so + sw],
                                start=start, stop=stop)
    with tile.TileContext(nc) as tc:
        with (
            tc.tile_pool(name="const", bufs=1) as constp,
            tc.tile_pool(name="dram", bufs=1, space="DRAM") as dramp,
        ):
            wsb = {}
            for nm, h in [("w1r", w1r), ("w1i", w1i), ("w1in", w1in),
                          ("w2r", w2r), ("w2i", w2i), ("w2in", w2in)]:
                t = constp.tile([BS, NBLK * BS], BF16, name=nm)
                nc.sync.dma_start(t[:], h[:])
                wsb[nm] = t
            bsb = constp.tile([BS, 8 * NBLK], F32)
            nc.sync.dma_start(bsb[:], bvec[:])
            wvb = constp.tile([BS, 4 * NBLK], F32)
            nc.sync.dma_start(wvb[:], wvec[:])
            vsb = constp.tile([66, 128], BF16)
            nc.sync.dma_start(vsb[:], vstack_h[:])
            msb = constp.tile([128, R * 64], BF16)
            nc.sync.dma_start(msb[:], mmats_h[:])

            ubuf = dramp.tile([2, C, FP], BF16)    # [re/im, c, f]
            u1buf = dramp.tile([128, R, C], BF16)  # [b0stack, k0, c]

            # ---- m broadcast to (96, F) via K=1 matmul ----
            onesb = constp.tile([1, BS], BF16)
            nc.vector.memset(onesb[:], 1.0)
            mbc = constp.tile([BS, F], BF16, name="mbc")
            with (
                tc.tile_pool(name="mvstage", bufs=1) as mvp,
                tc.tile_pool(name="psm", bufs=2, space="PSUM") as psm,
            ):
                mvsb = mvp.tile([1, F], BF16, name="mvs")
                nc.sync.dma_start(mvsb[:], mvs[0, :])
                for o, w in MCH:
                    pst = psm.tile([BS, 1024], F32, name="psb")
                    _mm(nc, pst, onesb[:], mvsb[:], o, w, True, True)
                    nc.scalar.copy(mbc[:, o:o + w], pst[:, 0:w])

            # ---- zero the spectrum pad f in [2049, 2112) in one DMA ----
            zpt = constp.tile([128, (FP - F) * 2 * C // 128], BF16)
            nc.vector.memset(zpt[:], 0.0)
            zpad_dma = nc.sync.dma_start(ubuf[:, :, F:FP], zpt[:])
            ub5 = ubuf[:].rearrange("h c (k2 k0) -> h k2 c k0", k0=R)
            u2 = ubuf[:].rearrange("h c f -> c h f")
            funnels = {}

            with (
                tc.tile_pool(name="xin", bufs=2) as xinp,
                tc.tile_pool(name="ps1", bufs=2, space="PSUM") as ps1p,
                tc.tile_pool(name="o1", bufs=2) as o1p,
                tc.tile_pool(name="sbu", bufs=2) as sbup,
                tc.tile_pool(name="cmb", bufs=1) as cmbp,
                tc.tile_pool(name="uo", bufs=2) as uop,
                tc.tile_pool(name="i1r", bufs=3) as i1rp,
                tc.tile_pool(name="i1ps", bufs=2, space="PSUM") as i1ps,
                tc.tile_pool(name="us", bufs=2) as usp,
            ):
                def emit_i1_group(cg):
                    us = usp.tile([128, R * 64], BF16, name="us")  # (k0, c64)
                    us3 = us[:].rearrange("p (k c) -> p k c", c=64)
                    for hf in range(2):  # c sub-groups of 32
                        c0 = cg * 64 + hf * 32
                        rt = i1rp.tile([66, 2048], BF16, name="rt")
                        d1 = nc.sync.dma_start(
                            rt[0:33, :].rearrange("p (c k0) -> p c k0", c=32),
                            ub5[0, :, c0:c0 + 32, :])
                        d2 = nc.sync.dma_start(
                            rt[33:66, :].rearrange("p (c k0) -> p c k0", c=32),
                            ub5[1, :, c0:c0 + 32, :])
                        kn = (64 * cg + 63) // 96
                        add_dep_helper(d1.ins, funnels[kn].ins, sync=True,
                                       reason="i1-after-combine")
                        add_dep_helper(d2.ins, funnels[kn].ins, sync=True,
                                       reason="i1-after-combine")
                        for j in range(4):
                            s = hf * 4 + j
                            pst = i1ps.tile([128, 512], F32, name="i1p")
                            nc.tensor.matmul(pst[:], vsb[:],
                                             rt[:, j * 512:(j + 1) * 512],
                                             start=True, stop=True)
                            ps3 = pst[:].rearrange("p (c k) -> p k c", c=8)
                            if s % 2 == 0:
                                nc.vector.tensor_copy(
                                    us3[:, :, s * 8:(s + 1) * 8], ps3)
                            else:
                                nc.scalar.copy(
                                    us3[:, :, s * 8:(s + 1) * 8], ps3)
                    nc.sync.dma_start(u1buf[:, :, cg * 64:(cg + 1) * 64], us3)

                for k in range(NBLK):
                    ksl = slice(k * BS, (k + 1) * BS)
                    bv = lambda t: bsb[:, t * NBLK + k:t * NBLK + k + 1]
                    wv = lambda t: wvb[:, t * NBLK + k:t * NBLK + k + 1]
                    xt = xinp.tile([BS, 2 * F], BF16, name="xt")
                    nc.sync.dma_start(
                        xt[:].rearrange("p (h f) -> p h f", h=2),
                        xfT[:, ksl, :].rearrange("h c f -> c h f"))
                    xr = xt[:, 0:F]
                    xi = xt[:, F:2 * F]
                    o1r = o1p.tile([BS, F], BF16, name="o1r")
                    o1i = o1p.tile([BS, F], BF16, name="o1i")
                    sr = sbup.tile([BS, F], BF16, name="sr")
                    si = sbup.tile([BS, F], BF16, name="si")
                    for o, w in MCH:
                        p1r = ps1p.tile([BS, 1024], F32, name="pmm")
                        _mm(nc, p1r, wsb["w1r"][:, ksl], xr, o, w, True, False)
                        _mm(nc, p1r, wsb["w1in"][:, ksl], xi, o, w, False, True)
                        nc.scalar.activation(o1r[:, o:o + w], p1r[:, 0:w],
                                             ACTF.Relu, bias=bv(0))
                        p1i = ps1p.tile([BS, 1024], F32, name="pmm")
                        _mm(nc, p1i, wsb["w1r"][:, ksl], xi, o, w, True, False)
                        _mm(nc, p1i, wsb["w1i"][:, ksl], xr, o, w, False, True)
                        nc.scalar.activation(o1i[:, o:o + w], p1i[:, 0:w],
                                             ACTF.Relu, bias=bv(1))
                    for o, w in MCH:
                        p2r = ps1p.tile([BS, 1024], F32, name="pmm")
                        _mm(nc, p2r, wsb["w2r"][:, ksl], o1r, o, w, True, False)
                        _mm(nc, p2r, wsb["w2in"][:, ksl], o1i, o, w, False, True)
                        a1 = cmbp.tile([BS, 1024], BF16, name="a1")
                        nc.scalar.activation(a1[:, 0:w], p2r[:, 0:w], ACTF.Relu,
                                             bias=bv(2))
                        a2 = cmbp.tile([BS, 1024], BF16, name="a2")
                        nc.scalar.activation(a2[:, 0:w], p2r[:, 0:w], ACTF.Relu,
                                             scale=-1.0, bias=bv(3))
                        nc.vector.tensor_sub(sr[:, o:o + w], a1[:, 0:w],
                                             a2[:, 0:w])
                        p2i = ps1p.tile([BS, 1024], F32, name="pmm")
                        _mm(nc, p2i, wsb["w2r"][:, ksl], o1i, o, w, True, False)
                        _mm(nc, p2i, wsb["w2i"][:, ksl], o1r, o, w, False, True)
                        a3 = cmbp.tile([BS, 1024], BF16, name="a3")
                        nc.scalar.activation(a3[:, 0:w], p2i[:, 0:w], ACTF.Relu,
                                             bias=bv(4))
                        a4 = cmbp.tile([BS, 1024], BF16, name="a4")
                        nc.scalar.activation(a4[:, 0:w], p2i[:, 0:w], ACTF.Relu,
                                             scale=-1.0, bias=bv(5))
                        nc.vector.tensor_sub(si[:, o:o + w], a3[:, 0:w],
                                             a4[:, 0:w])

                    # ---- combine: u = s * xf^2 * (w + wh*m) ----
                    ubuf_dmas = []
                    for fo, fw in FCH:
                        fs = slice(fo, fo + fw)
                        sq1 = cmbp.tile([BS, 1056], BF16, name="sq1")
                        sq2 = cmbp.tile([BS, 1056], BF16, name="sq2")
                        nc.scalar.square(sq1[:, 0:fw], xr[:, fs])
                        nc.scalar.square(sq2[:, 0:fw], xi[:, fs])
                        x2r = cmbp.tile([BS, 1056], BF16, name="x2r")
                        nc.vector.tensor_sub(x2r[:, 0:fw], sq1[:, 0:fw],
                                             sq2[:, 0:fw])
                        x2h = cmbp.tile([BS, 1056], BF16, name="x2h")
                        nc.gpsimd.tensor_mul(x2h[:, 0:fw], xr[:, fs], xi[:, fs])
                        sr2 = cmbp.tile([BS, 1056], BF16, name="sr2")
                        nc.vector.tensor_scalar(sr2[:, 0:fw], sr[:, fs], 2.0,
                                                None, op0=ALU.mult)
                        si2 = cmbp.tile([BS, 1056], BF16, name="si2")
                        nc.vector.tensor_scalar(si2[:, 0:fw], si[:, fs], 2.0,
                                                None, op0=ALU.mult)
                        m1 = cmbp.tile([BS, 1056], BF16, name="m1")
                        nc.vector.tensor_mul(m1[:, 0:fw], sr[:, fs],
                                             x2r[:, 0:fw])
                        m2 = cmbp.tile([BS, 1056], BF16, name="m2")
                        nc.gpsimd.tensor_mul(m2[:, 0:fw], si2[:, 0:fw],
                                             x2h[:, 0:fw])
                        vr = cmbp.tile([BS, 1056], BF16, name="vr")
                        nc.vector.tensor_sub(vr[:, 0:fw], m1[:, 0:fw],
                                             m2[:, 0:fw])
                        m3 = cmbp.tile([BS, 1056], BF16, name="m3")
                        nc.vector.tensor_mul(m3[:, 0:fw], si[:, fs],
                                             x2r[:, 0:fw])
                        m4 = cmbp.tile([BS, 1056], BF16, name="m4")
                        nc.gpsimd.tensor_mul(m4[:, 0:fw], sr2[:, 0:fw],
                                             x2h[:, 0:fw])
                        vi = cmbp.tile([BS, 1056], BF16, name="vi")
                        nc.vector.tensor_add(vi[:, 0:fw], m3[:, 0:fw],
                                             m4[:, 0:fw])
                        gr = cmbp.tile([BS, 1056], BF16, name="gr")
                        nc.vector.tensor_scalar(gr[:, 0:fw], mbc[:, fs], wv(2),
                                                wv(0), op0=ALU.mult, op1=ALU.add)
                        gi = cmbp.tile([BS, 1056], BF16, name="gi")
                        nc.vector.tensor_scalar(gi[:, 0:fw], mbc[:, fs], wv(3),
                                                wv(1), op0=ALU.mult, op1=ALU.add)
                        uo = uop.tile([BS, 2 * 1056], BF16, name="uo")
                        ur = uo[:, 0:fw]
                        ui = uo[:, 1056:1056 + fw]
                        n1 = cmbp.tile([BS, 1056], BF16, name="n1")
                        nc.vector.tensor_mul(n1[:, 0:fw], vr[:, 0:fw],
                                             gr[:, 0:fw])
                        n2 = cmbp.tile([BS, 1056], BF16, name="n2")
                        nc.gpsimd.tensor_mul(n2[:, 0:fw], vi[:, 0:fw],
                                             gi[:, 0:fw])
                        nc.vector.tensor_sub(ur, n1[:, 0:fw], n2[:, 0:fw])
                        n3 = cmbp.tile([BS, 1056], BF16, name="n3")
                        nc.vector.tensor_mul(n3[:, 0:fw], vr[:, 0:fw],
                                             gi[:, 0:fw])
                        n4 = cmbp.tile([BS, 1056], BF16, name="n4")
                        nc.vector.tensor_mul(n4[:, 0:fw], vi[:, 0:fw],
                                             gr[:, 0:fw])
                        nc.vector.tensor_add(ui, n3[:, 0:fw], n4[:, 0:fw])
                        ec = 0 if fo == 0 else 2048 - fo
                        nc.vector.tensor_scalar(uo[:, ec:ec + 1],
                                                uo[:, ec:ec + 1], 0.5, None,
                                                op0=ALU.mult)
                        nc.vector.memset(uo[:, 1056 + ec:1056 + ec + 1], 0.0)
                        ubuf_dmas.append(nc.sync.dma_start(
                            u2[ksl, :, fs],
                            uo[:].rearrange("p (h f) -> p h f", h=2)[:, :, 0:fw]))
                    fn = nc.sync.nop()
                    deps = list(ubuf_dmas)
                    if k == 0:
                        deps.append(zpad_dma)
                    else:
                        deps.append(funnels[k - 1])
                    for d in deps:
                        add_dep_helper(fn.ins, d.ins, sync=True,
                                       reason="block funnel")
                    funnels[k] = fn
                    for cg in range(12):
                        if (64 * cg + 63) // 96 == k:
                            emit_i1_group(cg)

            # ---- stage I2: out rows 64*b1 + b0 ----
            tc.strict_bb_all_engine_barrier()
            u1r4 = u1buf[:].rearrange("(j p) k c -> j p k c", j=2)
            with (
                tc.tile_pool(name="i2r", bufs=4) as i2rp,
                tc.tile_pool(name="i2ps", bufs=4, space="PSUM") as i2ps,
            ):
                for qp in range(R // 2):
                    q0 = 2 * qp
                    rt = i2rp.tile([128, 2 * C], BF16, name="rt2")
                    for j in range(2):
                        nc.sync.dma_start(
                            rt[j * 64:(j + 1) * 64, :].rearrange(
                                "p (q c) -> p q c", q=2),
                            u1r4[j, q0:q0 + 2, :, :].rearrange(
                                "q k c -> k q c"))
                    ev = i2rp.tile([64, 2 * C], BF16, name="ev2")
                    for half in range(2):
                        q = q0 + half
                        pst = i2ps.tile([64, C], F32, name="i2p")
                        lhs = msb[:, q * 64:(q + 1) * 64]
                        rhs0 = rt[:, half * C:half * C + 512]
                        rhs1 = rt[:, half * C + 512:(half + 1) * C]
                        nc.tensor.matmul(pst[:, 0:512], lhs, rhs0,
                                         start=True, stop=True)
                        nc.tensor.matmul(pst[:, 512:768], lhs, rhs1,
                                         start=True, stop=True)
                        if half == 0:
                            nc.scalar.copy(ev[:, 0:C], pst[:])
                        else:
                            nc.vector.tensor_copy(ev[:, C:2 * C], pst[:])
                    nc.sync.dma_start(
                        outb[:, q0:q0 + 2, :],
                        ev[:].rearrange("p (h c) -> p h c", h=2))
    nc.compile()
    return nc


# ------------------------------------------------------------------ host glue
def _quantile_linear(a, q):
    # jnp.quantile default method="linear" over flattened array
    a = np.sort(a, axis=None)
    n = a.shape[0]
    pos = q * (n - 1)
    lo = int(np.floor(pos))
    hi = min(lo + 1, n - 1)
    frac = pos - lo
    return a[lo] * (1 - frac) + a[hi] * frac


def _prep_l2(cw, cwh, w1, b1, w2, b2, m_row):
    """Per-core launch-2 input map minus the xf tensors. m_row: (F,) in {0,1,2}."""
    sv_re = np.ones(F, np.float32)
    sv_re[0] = 0.5
    sv_re[2048] = 0.5
    sv_im = np.ones(F, np.float32)
    sv_im[0] = 0.0
    sv_im[2048] = 0.0
    w1r_ = np.ascontiguousarray(np.concatenate([w1[0, k] for k in range(NBLK)], axis=1))
    w1i_ = np.ascontiguousarray(np.concatenate([w1[1, k] for k in range(NBLK)], axis=1))
    w2r_ = np.ascontiguousarray(np.concatenate([w2[0, k] for k in range(NBLK)], axis=1))
    w2i_ = np.ascontiguousarray(np.concatenate([w2[1, k] for k in range(NBLK)], axis=1))
    bvec = np.zeros((BS, 8 * NBLK), np.float32)
    wvec = np.zeros((BS, 4 * NBLK), np.float32)
    for k in range(NBLK):
        bvec[:, 0 * NBLK + k] = b1[0, k]
        bvec[:, 1 * NBLK + k] = b1[1, k]
        bvec[:, 2 * NBLK + k] = b2[0, k] - LAMBD
        bvec[:, 3 * NBLK + k] = -b2[0, k] - LAMBD
        bvec[:, 4 * NBLK + k] = b2[1, k] - LAMBD
        bvec[:, 5 * NBLK + k] = -b2[1, k] - LAMBD
        bvec[:, 6 * NBLK + k] = b2[0, k] + LAMBD
        bvec[:, 7 * NBLK + k] = b2[1, k] + LAMBD
        sl = slice(k * BS, (k + 1) * BS)
        wvec[:, 0 * NBLK + k] = cw[sl, 0]
        wvec[:, 1 * NBLK + k] = cw[sl, 1]
        wvec[:, 2 * NBLK + k] = cwh[sl, 0]
        wvec[:, 3 * NBLK + k] = cwh[sl, 1]
    import ml_dtypes
    bf = lambda a: np.ascontiguousarray(a).astype(ml_dtypes.bfloat16)
    return {
        "mvs": bf(np.asarray(m_row, np.float32).reshape(1, F)),
        "w1r": bf(w1r_), "w1i": bf(w1i_), "w1in": bf(-w1i_),
        "w2r": bf(w2r_), "w2i": bf(w2i_), "w2in": bf(-w2i_),
        "bvec": bvec, "wvec": wvec,
    }


def _masks(en, thr):
    """en: (B,F) energies; returns m (B,F) in {0,1,2}."""
    med = np.sort(en, axis=1)[:, (F - 1) // 2][:, None]  # method="lower"
    nrg = (en / (med + 1e-6)).astype(np.float32)
    thr_high = _quantile_linear(nrg, thr)
    thr_low = _quantile_linear(nrg, LOW_Q)
    mask_high = (nrg > thr_high).astype(np.float32)
    mask_low = (nrg <= thr_low).astype(np.float32)
    return mask_high * (1.0 + mask_low)


def kernel(x, complex_weight, complex_weight_high, w1, b1, w2, b2,
           threshold_param):
    x = np.asarray(x, np.float32)
    cw = np.asarray(complex_weight, np.float32)
    cwh = np.asarray(complex_weight_high, np.float32)
    w1 = np.asarray(w1, np.float32)
    b1 = np.asarray(b1, np.float32)
    w2 = np.asarray(w2, np.float32)
    b2 = np.asarray(b2, np.float32)
    thr = float(np.asarray(threshold_param).reshape(-1)[0])

    if "l1" not in _CACHE:
        _CACHE["l1"] = _build_l1()
    in_maps1 = [{"x": np.ascontiguousarray(x[i])} for i in range(B)]
    _t0 = _time.time()
    res1 = run_bass_kernel_spmd(_CACHE["l1"], in_maps1, core_ids=list(range(B)),
                                trace=TRACE)
    _t1 = _time.time()
    r1 = res1.results if hasattr(res1, "results") else res1

    xfT, en = [], []
    for r in r1:
        full = np.zeros((66, 64, C), np.float32)
        full[:, 0:33] = np.asarray(r["xf_a"]).astype(np.float32)
        full[:, 33:64] = np.asarray(r["xf_b"]).astype(np.float32)[:, ::-1]
        # xfT[h, c, f]: h=0 Re, h=1 Im
        t = np.empty((2, C, F), np.float32)
        t[0] = full[0:33].reshape(33 * 64, C)[:F].T
        t[1] = full[33:66].reshape(33 * 64, C)[:F].T
        xfT.append(t)
        e = np.asarray(r["energy"])
        en.append((e[0:33] + e[33:66]).reshape(-1)[:F])
    en = np.stack(en)
    m = _masks(en, thr)

    if "l2" not in _CACHE:
        _CACHE["l2"] = _build_l2()
    import ml_dtypes
    in_maps2 = []
    for i in range(B):
        im = dict(_prep_l2(cw, cwh, w1, b1, w2, b2, m[i]))
        im["xfT"] = np.ascontiguousarray(xfT[i]).astype(ml_dtypes.bfloat16)
        in_maps2.append(im)
    _t2 = _time.time()
    res2 = run_bass_kernel_spmd(_CACHE["l2"], in_maps2, core_ids=list(range(B)),
                                trace=TRACE)
    _t3 = _time.time()
    r2 = res2.results if hasattr(res2, "results") else res2

    out = np.stack([np.asarray(r["outb"]).astype(np.float32).reshape(N, C)
                    for r in r2])
    LAST_NS.clear()
    for res in (res1, res2):
        LAST_NS.append(getattr(res, "exec_time_ns", None))
    LAST_NS.append(("wall_l1_s", _t1 - _t0))
    LAST_NS.append(("wall_l2_s", _t3 - _t2))
    return out.astype(np.float32)

